# revision 56
# baseline (speedup 1.0000x reference)
"""Dynamic depthwise-conv branch (DynamicConvBranch) Trainium2 kernel.

Problem (hardcoded shapes):
  x  [16, 32, 384, 384] f32
  w1 [32, 128], b1 [128], w2 [128, 288], b2 [288]
  out[b,c] = conv2d_same3x3(x[b,c], k[b,c]) where
  k = reshape(relu(mean_hw(x) @ w1 + b1) @ w2 + b2, [B, 32, 3, 3])

Strategy: pure data parallel over batch (2 samples per core, 8 cores).
x and the output are staged to bf16 on the host, and x gets one zero
row of padding top+bottom (halves DMA traffic; rel-err budget is 2e-2,
measured end-to-end error ~5.7e-3).  Per sample, x is resident in SBUF
as 4 row-strips x 4 channel-groups of [98, 8, 386] bf16 tiles, one
uniform tile class whose halo rows load straight from the padded HBM
image.  Pipeline (cost-model 168.5us vs 203.7us for the session-start
baseline):

 * Head (~33us, DMA-bound): 16 x-tile loads own the SP/HWDGE path;
   consts ride Pool's SWDGE so their DMA-FIFO slots interleave without
   perturbing the x stream.  Pooling chases the loads at the 1.66us
   DMA cadence: DVE takes 6 channels per tile (255ns/op cadence) and
   ACT two, except the last tiles which keep ACT clear for the MLP;
   the very last tile loads as two half-group DMAs so its pooling
   overlaps its own transfer.  Strip-sums contract csum.T @ onesp per
   group (zeroed halo weights drop the overlap rows) and h1
   accumulates per group against base-0 w1 slices as each group's
   means land, so after the last pool only g3's chain remains.
   Short pm-gated filler matmuls keep PE continuously busy through
   the chain's ACT hops, so the p-state ramp finishes near conv start.
 * Conv (PE-bound, >98% engine occupancy): band matrices build on DVE
   in 5 wide TensorTensor ops per group (first group per-channel for
   minimum first-matmul latency).  The 3x3 depthwise conv is 12
   PSUM-accumulated bf16 matmuls per (channel, strip-pair) with dw
   OUTER so each band slice serves two consecutive matmuls (half the
   Ldweights).  ACT drains PSUM into bf16 staging; stores go out one
   channel per DMA from Pool's SWDGE (384 descriptors keeps two
   in-flight inside the 1024-descriptor ring, so issue never blocks).
 * Sample 1 overlap: its loads start the moment sample 0's land
   (g2/g3 on fresh slots, g0/g1 gated on conv0 slot release), its
   pooling fills DVE slack between band builds, and MLP1 is emitted
   between conv0-g2 and conv0-g3 so PE runs it hot mid-conv -- no
   bubble between the two conv windows.
 * Tail (~4.5us): the final channel drains and stores per strip, the
   last three stores on three different issue paths (Pool SWDGE, SP
   HWDGE, ACT HWDGE) so descriptor generation runs in parallel.
"""

import numpy as np

B, C, H, W = 16, 32, 384, 384
NK = 32
HID = 128
KK = 3
N_CORES = 8
B_PER_CORE = B // N_CORES

GC = 8           # channels per DMA group
NG = C // GC     # 4 groups
SH = 96          # output rows per strip
NS = H // SH     # 4 strips
KP = SH + 2      # tile partitions (1-row halo each side) = 98
WP = W + 2       # padded width: cols 0 and 385 are zero
XB_X = 25        # x tile slots (16 = one sample fully resident)

_CACHE = {}


def _build_nc():
    from contextlib import ExitStack
    from concourse import bass, bacc, tile
    from concourse.bass import mybir
    import ml_dtypes

    f32 = mybir.dt.float32
    bf16 = mybir.dt.bfloat16
    Alu = mybir.AluOpType
    Act = mybir.ActivationFunctionType

    nc = bacc.Bacc()

    # x arrives host-padded with one zero row top and bottom [H+2 rows], so
    # every strip (including edge strips) loads its full 98-row window
    # straight from HBM -- no on-chip halo zeroing, one uniform tile class
    x_d = nc.dram_tensor("x", [B_PER_CORE, C, H + 2, W], bf16,
                         kind="ExternalInput")
    w1_d = nc.dram_tensor("w1", [C, HID], f32, kind="ExternalInput")
    b1_d = nc.dram_tensor("b1", [HID], f32, kind="ExternalInput")
    w2_d = nc.dram_tensor("w2", [HID, NK * KK * KK], f32, kind="ExternalInput")
    b2_d = nc.dram_tensor("b2", [NK * KK * KK], f32, kind="ExternalInput")
    out_d = nc.dram_tensor("out", [B_PER_CORE, NK, H, W], bf16,
                           kind="ExternalOutput")

    # Host-baked diagonal masks replicated over dw: masks3[p, dh, m, dw] = 1
    # iff p == m + dh.  Band matrices for a whole channel-group build as
    # A3[p, c, m, dw] = sum_dh masks3[p, dh, m, dw] * k[c, dh, dw] with five
    # wide TensorTensor ops (the dw-last layout keeps every operand's last
    # AP dim packed, so DVE runs them in 2x mode).
    masks_np = np.zeros((KP, KK, SH, KK), dtype=np.float32)
    for dh in range(KK):
        for m in range(SH):
            masks_np[m + dh, dh, m, :] = 1.0
    masks_d = nc.inline_tensor(masks_np.astype(ml_dtypes.bfloat16),
                               name="bandmasks")
    # strip-sum weights: drop the halo rows (compute ops must start at
    # partition 0, so pooling reads all 98 rows and the contraction masks)
    onesp_np = np.ones((KP, 1), dtype=np.float32)
    onesp_np[0, 0] = 0.0
    onesp_np[KP - 1, 0] = 0.0
    onesp_d = nc.inline_tensor(onesp_np, name="onesp")

    with tile.TileContext(nc) as tc, ExitStack() as ctx:
        xpool = ctx.enter_context(tc.tile_pool(name="xp", bufs=XB_X))
        dpool = ctx.enter_context(tc.tile_pool(name="dump", bufs=2))
        dpool_a = ctx.enter_context(tc.tile_pool(name="dumpa", bufs=1))
        cpool = ctx.enter_context(tc.tile_pool(name="const", bufs=1))
        mpool = ctx.enter_context(tc.tile_pool(name="mlp", bufs=2))
        apool = ctx.enter_context(tc.tile_pool(name="amat", bufs=3))
        tpool = ctx.enter_context(tc.tile_pool(name="atmp", bufs=2))
        cspool = ctx.enter_context(tc.tile_pool(name="csum", bufs=NS * NG + 2))
        opool = ctx.enter_context(tc.tile_pool(name="ostage", bufs=3))
        pp_ps = ctx.enter_context(
            tc.tile_pool(name="poolps", bufs=1, space=bass.MemorySpace.PSUM))
        kb_ps_pool = ctx.enter_context(
            tc.tile_pool(name="kbps", bufs=1, space=bass.MemorySpace.PSUM))
        cv_ps = ctx.enter_context(
            tc.tile_pool(name="convps", bufs=3, space=bass.MemorySpace.PSUM))

        # --- one-time constants ---
        # (tiles declared here; the DMAs are emitted after the first x group
        # so the x pipeline owns the DMA engines from t=0.  Earliest consts
        # use: onesp at the first strip-sum ~9us, the rest ~29us.)
        masks = cpool.tile([KP, KK, SH, KK], bf16)
        ones1 = cpool.tile([1, HID], f32)         # for partition broadcast
        nc.vector.memset(ones1[:], 1.0)
        onesp = cpool.tile([KP, 1], f32)          # strip-sum, halo rows zeroed
        # w1 is loaded once per channel group at partition base 0, so the h1
        # accumulation can consume each group's pooled means independently
        w1g = [cpool.tile([GC, HID], f32, name=f"w1g{g}") for g in range(NG)]
        b1row = cpool.tile([1, HID], f32)
        w2s = cpool.tile([HID, NK * KK * KK], f32)
        b2row = cpool.tile([1, NK * KK * KK], f32)

        def emit_const_loads_early():
            # after g0's x loads: 2 HWDGE slots; the DMA backlog absorbs it
            nc.sync.dma_start(onesp[:], onesp_d[:])
            nc.sync.dma_start(masks[:], masks_d[:])

        def emit_const_loads_late():
            # after g2's x loads, via Pool's SWDGE: the DMA FIFO slots these
            # behind the queued x transfers (they land ~29us, just before
            # the MLP consumes them) and the x pipeline's SP/HWDGE path is
            # never perturbed
            for g in range(NG):
                nc.gpsimd.dma_start(w1g[g][:], w1_d[g * GC:(g + 1) * GC, :])
            nc.gpsimd.dma_start(b1row[:], b1_d[:].unsqueeze(0))
            nc.gpsimd.dma_start(w2s[:], w2_d[:])
            nc.gpsimd.dma_start(b2row[:], b2_d[:].unsqueeze(0))

        # tiny warm-up matmul: absorbs the PE preamble wait + const DMA lane
        # ticks so real matmuls carry few semaphore waits (ISA slot limit).
        warm_ps = pp_ps.tile([1, 1], f32, tag="pool")
        nc.tensor.matmul(warm_ps[:], ones1[0:1, 0:1], ones1[0:1, 0:1],
                         start=True, stop=True)

        # x slots are zero-padded (cols 0/385) on first use; loads never
        # touch those columns, so slot rotation keeps them zero.  Halo rows
        # come from the host-side zero padding of x.
        x_acq = [0]

        def x_tile():
            t = xpool.tile([KP, GC, WP], bf16, tag="x")
            if x_acq[0] < XB_X:
                nc.gpsimd.memset(t[:, :, 0:1], 0.0)
                nc.gpsimd.memset(t[:, :, WP - 1:WP], 0.0)
            x_acq[0] += 1
            return t

        xt = {}      # (b, g, s) -> x tile
        csum = {}    # (b, g, s) -> [96, GC] f32 row sums

        def emit_load(b, g, split_last=False):
            """Load DMAs for one channel-group of sample b.  In the padded
            row coordinates, strip s covers rows [s*SH, s*SH + KP).  The
            head's very last tile is loaded as two half-group DMAs so its
            pooling overlaps the second half's transfer."""
            c0 = g * GC
            xsrc = x_d[b, c0:c0 + GC]
            for s in range(NS):
                t = x_tile()
                if split_last and s == NS - 1:
                    for h in range(2):
                        nc.sync.dma_start(
                            t[:, 4 * h:4 * h + 4, 1:W + 1],
                            xsrc[4 * h:4 * h + 4,
                                 s * SH:s * SH + KP, :].rearrange(
                                     "c r w -> r c w"))
                else:
                    nc.sync.dma_start(
                        t[:, :, 1:W + 1],
                        xsrc[:, s * SH:s * SH + KP, :].rearrange(
                            "c r w -> r c w"))
                xt[(b, g, s)] = t

        def pool_tile(b, g, s, split=None):
            """W-sums of one tile: tensor_scalar with the per-partition
            accumulator (160ns engine / ~255ns cadence per channel on DVE).
            Reads all 98 rows (partition offset must be 0); halo rows are
            dropped later by the onesp strip-sum contraction.  `split` maps
            engines to channel lists: in the head, DVE alone (2.04us/tile)
            cannot keep up with the 1.66us DMA cadence, so Pool and ACT
            each take a channel."""
            t = xt[(b, g, s)]
            cs = cspool.tile([KP, GC], f32, tag="cs")
            if split is None:
                split = [("dve", range(GC))]
            for eng, ccs in split:
                if not ccs:
                    continue
                if eng == "dve":
                    dump = dpool.tile([KP, W], bf16, tag="dump")
                    for cc in ccs:
                        nc.vector.tensor_scalar(
                            dump[:], t[:, cc, 1:W + 1], 1.0, 0.0,
                            op0=Alu.mult, op1=Alu.add,
                            accum_out=cs[:, cc:cc + 1])
                else:
                    dump = dpool_a.tile([KP, W], bf16, tag="dumpa",
                                        name="dumpa")
                    for cc in ccs:
                        nc.scalar.activation(dump[:], t[:, cc, 1:W + 1],
                                             Act.Copy,
                                             accum_out=cs[:, cc:cc + 1])
            csum[(b, g, s)] = cs

        def emit_mlp(b, warm=8):
            """Strip-sum matmuls + kernel-generator MLP (f32); returns kb.
            Chain only touches PE and ACT (DVE may be backlogged).  The
            strip-sums contract csum.T @ onesp so the channel means land as
            a COLUMN directly (no transpose round-trip).  For sample 0,
            pm-gated warm-up matmuls are interleaved into the chain's ACT
            gaps so PE stays continuously busy from the pm step through the
            first conv matmul -- by then the p-state ramp (3us) is done."""
            mlpx = mpool.tile([HID, 1 + NG], f32, tag="mlpx")
            h1s = mlpx[:, 0:1]
            pmg = mlpx[0:GC, 1:1 + NG]     # per-group pooled means, base 0
            pcol_ps = pp_ps.tile([GC, NG], f32, tag="pool")
            for g in range(NG):
                for s in range(NS):
                    # csum.T @ onesp: the group's channel sums land as a
                    # COLUMN at partition base 0 (no transpose round-trip)
                    nc.tensor.matmul(pcol_ps[0:GC, g:g + 1],
                                     csum[(b, g, s)][:], onesp[:],
                                     start=(s == 0), stop=(s == NS - 1))
                nc.scalar.activation(pmg[:, g:g + 1], pcol_ps[0:GC, g:g + 1],
                                     Act.Copy, scale=1.0 / (H * W))

            def warm2(n, cols=144):
                # p-state filler on the otherwise-idle kbps bank; gated on
                # the LAST group's means so it runs exactly in the chain's
                # ACT-hop gaps (not earlier, during the load phase).  Short
                # (144-col) fillers so an overshoot never delays the chain.
                if b != 0:
                    return
                for _ in range(n):
                    wp = kb_ps_pool.tile([1, NK * KK * KK], f32, tag="kbps",
                                         name="warm2")
                    nc.tensor.matmul(wp[:, 0:cols], pmg[0:1, NG - 1:NG],
                                     w2s[0:1, 0:cols], start=True, stop=True)

            h1_ps = pp_ps.tile([HID, 1], f32, tag="pool")
            for g in range(NG):
                nc.tensor.matmul(h1_ps[:], w1g[g][:], pmg[:, g:g + 1],
                                 start=(g == 0), stop=False)
            nc.tensor.matmul(h1_ps[:], b1row[:], ones1[0:1, 0:1],
                             start=False, stop=True)
            warm2(3)
            nc.scalar.activation(h1s, h1_ps[:], Act.Relu)

            k_ps = pp_ps.tile([1, NK * KK * KK], f32, tag="pool")
            nc.tensor.matmul(k_ps[:], h1s, w2s[:], start=True, stop=True)
            warm2(3)
            krow = mpool.tile([1, NK * KK * KK], f32, tag="krow")
            nc.scalar.activation(krow[:], k_ps[:], Act.Copy)

            kb = kb_ps_pool.tile([HID, NK * KK * KK], f32, tag="kbps")
            nc.tensor.matmul(kb[:], ones1[:], krow[:], start=True, stop=False)
            nc.tensor.matmul(kb[:], ones1[:], b2row[:], start=False, stop=True)
            if b == 0:
                for _ in range(3):
                    wp = pp_ps.tile([1, NK * KK * KK], f32, tag="pool",
                                    name="warm3")
                    nc.tensor.matmul(wp[:, 0:144], krow[0:1, 0:1],
                                     krow[0:1, 0:144], start=True, stop=True)
            kbs = mpool.tile([HID, NK * KK * KK], bf16, tag="kbs")
            # group-0's slice first: the first band build gates on only this
            nc.scalar.activation(kbs[:, 0:GC * KK * KK],
                                 kb[:, 0:GC * KK * KK], Act.Copy)
            nc.scalar.activation(kbs[:, GC * KK * KK:],
                                 kb[:, GC * KK * KK:], Act.Copy)
            # gated warm-up: PE idled through pooling, so its p-state clock
            # dropped; back-to-back dummy matmuls (first ready only once
            # kbs exists) ramp it back to full speed exactly while DVE builds
            # the first band matrices, so conv starts at 2.4 GHz.  Sample 1's
            # MLP runs mid-conv0 with PE already hot, so warm=0 there.
            for _ in range(warm):
                w_ps = pp_ps.tile([1, NK * KK * KK], f32, tag="pool",
                                  name="w_ps")
                nc.tensor.matmul(w_ps[:], kbs[0:1, 0:1], kbs[0:1, :],
                                 start=True, stop=True)
            return kbs

        def emit_amat_group(kbs, g, halves=1):
            """Band matrices for a whole 8-channel group in 5 wide DVE
            TensorTensor ops (2x mode: every operand's last AP dim is the
            packed dw axis): A3[p, c, m, dw] = sum_dh masks3 * k[c, dh, dw].
            halves=2 splits the build in two for lower first-channel latency.
            """
            a3 = apool.tile([KP, GC, SH, KK], bf16, tag="a3", name="a3")
            # [98, 8ch, 9] view of this group's kernel block
            gb = kbs[0:KP, g * GC * 9:(g + 1) * GC * 9].rearrange(
                "p (c z) -> p c z", c=GC)
            hc = GC // halves
            for h in range(halves):
                c0, c1 = h * hc, (h + 1) * hc
                for dh in range(KK):
                    m_b = masks[:, dh].unsqueeze(1).broadcast_to(
                        [KP, hc, SH, KK])
                    k_b = gb[:, c0:c1, dh * KK:(dh + 1) * KK].unsqueeze(2) \
                        .broadcast_to([KP, hc, SH, KK])
                    if dh == 0:
                        nc.vector.tensor_tensor(a3[:, c0:c1], m_b, k_b,
                                                Alu.mult)
                    else:
                        t = tpool.tile([KP, hc, SH, KK], bf16, tag="at",
                                       name="at")
                        nc.vector.tensor_tensor(t[:], m_b, k_b, Alu.mult)
                        nc.vector.tensor_tensor(a3[:, c0:c1], a3[:, c0:c1],
                                                t[:], Alu.add)
            return a3

        def emit_conv_channel(b, c, a3, ob):
            """12 PSUM-accumulated matmuls + drains for channel c.
            dw is the OUTER loop within a strip pair so the band-matrix
            weight slice is reused by two consecutive matmuls: the PE
            sequencer emits half the Ldweights (interleaved accumulation on
            two PSUM banks is fine -- accumulation state lives in PSUM)."""
            g, cc = divmod(c, GC)
            last = (b == B_PER_CORE - 1 and c == NK - 1)
            for j in range(NS // 2):
                o_ps = cv_ps.tile([SH, 2, 512], f32, tag="cv")  # 2 banks
                for dw in range(KK):
                    for half in range(2):
                        s = 2 * j + half
                        t = xt[(b, g, s)]
                        nc.tensor.matmul(o_ps[:, half, 0:W],
                                         a3[:, cc, :, dw],
                                         t[:, cc, dw:dw + W],
                                         start=(dw == 0), stop=(dw == KK - 1))
                # PSUM drains all on ACT (GpSimd cannot access PSUM on
                # HW; DVE carries every band-matrix build and all pooling)
                if last:
                    # final channel of the kernel: drain and store per STRIP
                    # so the critical tail is one [96,384] drain + one small
                    # store.  Strip 2's drain runs on DVE so ACT's in-order
                    # queue reaches the final strip-3 drain immediately, and
                    # the last three stores go out on three DIFFERENT issue
                    # paths (Pool SWDGE / SP HWDGE / ACT HWDGE) so their
                    # descriptor generation runs in parallel.
                    engs = [nc.gpsimd, nc.gpsimd, nc.sync, nc.scalar]
                    for half in range(2):
                        s = 2 * j + half
                        drain = nc.vector.tensor_scalar if s == 2 else None
                        if drain is not None:
                            nc.vector.tensor_scalar(
                                ob[:, c % 2, s:s + 1, :],
                                o_ps[:, half:half + 1, 0:W],
                                1.0, 0.0, op0=Alu.mult, op1=Alu.add)
                        else:
                            nc.scalar.activation(ob[:, c % 2, s:s + 1, :],
                                                 o_ps[:, half:half + 1, 0:W],
                                                 Act.Copy)
                        engs[s].dma_start(
                            out_d[b, c, s * SH:(s + 1) * SH, :].rearrange(
                                "(s2 p) w -> p s2 w", s2=1),
                            ob[:, c % 2, s:s + 1])
                else:
                    nc.scalar.activation(ob[:, c % 2, 2 * j:2 * j + 2, :],
                                         o_ps[:, :, 0:W], Act.Copy)
            # one store per channel, issued from the idle Pool engine's
            # SWDGE path: 384 descriptors per store keeps two stores inside
            # the 1024-descriptor SWDGE ring, so issue never blocks on a
            # prior transfer, and the tail after the last drain is short
            if not last:
                nc.gpsimd.dma_start(
                    out_d[b, c, :, :].rearrange("(s p) w -> p s w", s=NS),
                    ob[:, c % 2])

        # ---------------- schedule ----------------
        # head: load + pool sample 0.  ALL pooling runs on DVE (1.28us per
        # tile vs the 1.66us DMA cadence, so DVE keeps pace): ACT's in-order
        # queue stays empty so the MLP chain's ACT hops run the moment each
        # group's strip-sums land.
        for g in range(NG):
            emit_load(0, g, split_last=(g == NG - 1))
            if g == 0:
                emit_const_loads_early()
            elif g == 1:
                # early enough that their DMA-FIFO slots interleave with the
                # head x transfers: every const lands (+sem) by ~27us, ahead
                # of the MLP chain's first use at ~30.5us
                emit_const_loads_late()
            for s in range(NS):
                if (g, s) == (NG - 1, NS - 1):
                    # very last tile: first half pools on DVE while the
                    # second half transfers; ACT (idle since ~26us) takes
                    # two channels of the second half
                    pool_tile(0, g, s, split=[("dve", range(6)),
                                              ("act", [6, 7])])
                elif (g, s) == (NG - 1, NS - 2):
                    # all-DVE so DVE reaches the last tile's pools promptly
                    pool_tile(0, g, s)
                else:
                    pool_tile(0, g, s, split=[("dve", range(6)),
                                              ("act", [6, 7])])
        kb0 = emit_mlp(0)

        # conv sample 0; sample 1's loads go out immediately in order
        # g2,g3,g0,g1 (g2/g3 land on fresh slots, g0/g1 self-gate on slot
        # reuse after conv0's matmuls release the sample-0 slots).
        for g in (2, 3, 0, 1):
            emit_load(1, g)

        _ob_cur = {}

        def _conv_ch(b, c, a3):
            if c % 2 == 0:
                _ob_cur["t"] = opool.tile([SH, 2, NS, W], bf16, tag="ob",
                                          name="ob")
            emit_conv_channel(b, c, a3, _ob_cur["t"])

        def conv_group(b, kbs, g, halves=1):
            a3 = emit_amat_group(kbs, g, halves=halves)
            for cc in range(GC):
                _conv_ch(b, g * GC + cc, a3)

        # Band-matrix builds are the latency-critical DVE work (each conv
        # group's a3 must exist before PE reaches that group), so they are
        # emitted EAGERLY -- back-to-back where slots allow (apool holds 3)
        # -- and sample 1's pooling fills the DVE slack between them.
        # MLP1 is emitted BETWEEN conv0-g2 and conv0-g3: PE runs it
        # mid-conv with the p-state hot, and DVE builds sample 1's first
        # band matrices during conv0-g3 -- no PE bubble between samples.
        a3_01 = [emit_amat_group(kb0, 0, halves=8),
                 emit_amat_group(kb0, 1, halves=2)]
        for cc in range(GC):
            _conv_ch(0, 0 * GC + cc, a3_01[0])
        for s in range(NS):
            pool_tile(1, 2, s)
        a3_2 = emit_amat_group(kb0, 2, halves=2)
        for cc in range(GC):
            _conv_ch(0, 1 * GC + cc, a3_01[1])
        for s in range(NS):
            pool_tile(1, 3, s)
        a3_3 = emit_amat_group(kb0, 3, halves=2)
        for cc in range(GC):
            _conv_ch(0, 2 * GC + cc, a3_2)
        for g in (0, 1):
            for s in range(NS):
                pool_tile(1, g, s)
        kb1 = emit_mlp(1, warm=0)
        for cc in range(GC):
            _conv_ch(0, 3 * GC + cc, a3_3)
        for g in range(NG):
            conv_group(1, kb1, g, halves=2)

    nc.compile()
    return nc


def _make_exec():
    """Build + jit the SPMD executable once; returns a callable over numpy inputs."""
    import jax
    from jax.sharding import Mesh, PartitionSpec
    from jax.experimental.shard_map import shard_map
    from concourse import bass2jax
    import concourse.mybir as mybir

    nc = _build_nc()
    _CACHE["nc"] = nc
    bass2jax.install_neuronx_cc_hook()

    in_names, out_names, out_shapes, out_dtypes = [], [], [], []
    for alloc in nc.m.functions[0].allocations:
        if not isinstance(alloc, mybir.MemoryLocationSet):
            continue
        name = alloc.memorylocations[0].name
        if alloc.kind == "ExternalInput":
            in_names.append(name)
        elif alloc.kind == "ExternalOutput":
            out_names.append(name)
            out_shapes.append(tuple(alloc.tensor_shape))
            out_dtypes.append(mybir.dt.np(alloc.dtype))
    partition_name = nc.partition_id_tensor.name if nc.partition_id_tensor else None
    if partition_name in in_names:
        in_names.remove(partition_name)
    n_params = len(in_names)
    out_avals = [jax.core.ShapedArray(s, d) for s, d in zip(out_shapes, out_dtypes)]
    all_names = in_names + out_names
    if partition_name is not None:
        all_names = all_names + [partition_name]
    donate = tuple(range(n_params, n_params + len(out_names)))

    def _body(*args):
        operands = list(args)
        if partition_name is not None:
            operands.append(bass2jax.partition_id_tensor())
        outs = bass2jax._bass_exec_p.bind(
            *operands,
            out_avals=tuple(out_avals),
            in_names=tuple(all_names),
            out_names=tuple(out_names),
            lowering_input_output_aliases=(),
            sim_require_finite=True,
            sim_require_nnan=True,
            nc=nc,
        )
        return tuple(outs)

    devices = jax.devices()[:N_CORES]
    mesh = Mesh(np.asarray(devices), ("core",))
    in_specs = (PartitionSpec("core"),) * (n_params + len(out_names))
    out_specs = (PartitionSpec("core"),) * len(out_names)
    sharded = jax.jit(
        shard_map(_body, mesh=mesh, in_specs=in_specs, out_specs=out_specs,
                  check_rep=False),
        donate_argnums=donate, keep_unused=True)

    def run(in_maps):
        concat_in = [
            np.concatenate([np.asarray(in_maps[c][nm]) for c in range(N_CORES)], axis=0)
            for nm in in_names
        ]
        concat_zeros = [
            np.zeros((N_CORES * s[0], *s[1:]), d)
            for s, d in zip(out_shapes, out_dtypes)
        ]
        out_arrs = sharded(*concat_in, *concat_zeros)
        out_arrs = jax.block_until_ready(out_arrs)
        return {nm: np.asarray(out_arrs[i]) for i, nm in enumerate(out_names)}

    return run


def _run(inputs, trace=False):
    import ml_dtypes
    if "exec" not in _CACHE:
        _CACHE["exec"] = _make_exec()
    run = _CACHE["exec"]

    x16 = np.ascontiguousarray(inputs["x"]).astype(ml_dtypes.bfloat16)
    # one zero row of padding top and bottom: edge strips load their halo
    # rows straight from HBM like every other strip
    x16 = np.pad(x16, ((0, 0), (0, 0), (1, 1), (0, 0)))
    in_maps = []
    for i in range(N_CORES):
        in_maps.append({
            "x": x16[i * B_PER_CORE:(i + 1) * B_PER_CORE],
            "w1": inputs["w1"], "b1": inputs["b1"],
            "w2": inputs["w2"], "b2": inputs["b2"],
        })
    outs = run(in_maps)
    out = outs["out"].reshape(B, NK, H, W).astype(np.float32)
    return out, None


def kernel(**inputs):
    out, _ = _run(inputs, trace=False)
    return out



# revision 62
# speedup vs baseline: 1.0133x; 1.0133x over previous
"""Dynamic depthwise-conv branch (DynamicConvBranch) Trainium2 kernel.

Problem (hardcoded shapes):
  x  [16, 32, 384, 384] f32
  w1 [32, 128], b1 [128], w2 [128, 288], b2 [288]
  out[b,c] = conv2d_same3x3(x[b,c], k[b,c]) where
  k = reshape(relu(mean_hw(x) @ w1 + b1) @ w2 + b2, [B, 32, 3, 3])

Strategy: pure data parallel over batch (2 samples per core, 8 cores).
x and the output are staged to bf16 on the host, and x gets one zero
row of padding top+bottom (halves DMA traffic; rel-err budget is 2e-2,
measured end-to-end error ~5.7e-3).  Per sample, x is resident in SBUF
as 4 row-strips x 4 channel-groups of [98, 8, 386] bf16 tiles, one
uniform tile class whose halo rows load straight from the padded HBM
image.  Pipeline (cost-model 168.5us vs 203.7us for the session-start
baseline):

 * Head (~33us, DMA-bound): 16 x-tile loads own the SP/HWDGE path;
   consts ride Pool's SWDGE so their DMA-FIFO slots interleave without
   perturbing the x stream.  Pooling chases the loads at the 1.66us
   DMA cadence: DVE takes 6 channels per tile (255ns/op cadence) and
   ACT two, except the last tiles which keep ACT clear for the MLP;
   the very last tile loads as two half-group DMAs so its pooling
   overlaps its own transfer.  Strip-sums contract csum.T @ onesp per
   group (zeroed halo weights drop the overlap rows) and h1
   accumulates per group against base-0 w1 slices as each group's
   means land, so after the last pool only g3's chain remains.
   Short pm-gated filler matmuls keep PE continuously busy through
   the chain's ACT hops, so the p-state ramp finishes near conv start.
 * Conv (PE-bound, >98% engine occupancy): band matrices build on DVE
   in 5 wide TensorTensor ops per group (first group per-channel for
   minimum first-matmul latency).  The 3x3 depthwise conv is 12
   PSUM-accumulated bf16 matmuls per (channel, strip-pair) with dw
   OUTER so each band slice serves two consecutive matmuls (half the
   Ldweights).  ACT drains PSUM into bf16 staging; stores go out one
   channel per DMA from Pool's SWDGE (384 descriptors keeps two
   in-flight inside the 1024-descriptor ring, so issue never blocks).
 * Sample 1 overlap: its loads start the moment sample 0's land
   (g2/g3 on fresh slots, g0/g1 gated on conv0 slot release), its
   pooling fills DVE slack between band builds, and MLP1 is emitted
   between conv0-g2 and conv0-g3 so PE runs it hot mid-conv -- no
   bubble between the two conv windows.
 * Tail (~4.5us): the final channel drains and stores per strip, the
   last three stores on three different issue paths (Pool SWDGE, SP
   HWDGE, ACT HWDGE) so descriptor generation runs in parallel.
"""

import numpy as np

B, C, H, W = 16, 32, 384, 384
NK = 32
HID = 128
KK = 3
N_CORES = 8
B_PER_CORE = B // N_CORES

GC = 8           # channels per DMA group
NG = C // GC     # 4 groups
SH = 96          # output rows per strip
NS = H // SH     # 4 strips
KP = SH + 2      # tile partitions (1-row halo each side) = 98
WP = W + 2       # padded width: cols 0 and 385 are zero
XB_X = 25        # x tile slots (16 = one sample fully resident)

_CACHE = {}


def _build_nc():
    from contextlib import ExitStack
    from concourse import bass, bacc, tile
    from concourse.bass import mybir
    import ml_dtypes

    f32 = mybir.dt.float32
    bf16 = mybir.dt.bfloat16
    Alu = mybir.AluOpType
    Act = mybir.ActivationFunctionType

    nc = bacc.Bacc()

    # x arrives host-padded with one zero row top and bottom [H+2 rows], so
    # every strip (including edge strips) loads its full 98-row window
    # straight from HBM -- no on-chip halo zeroing, one uniform tile class
    x_d = nc.dram_tensor("x", [B_PER_CORE, C, H + 2, W], bf16,
                         kind="ExternalInput")
    w1_d = nc.dram_tensor("w1", [C, HID], f32, kind="ExternalInput")
    b1_d = nc.dram_tensor("b1", [HID], f32, kind="ExternalInput")
    w2_d = nc.dram_tensor("w2", [HID, NK * KK * KK], f32, kind="ExternalInput")
    b2_d = nc.dram_tensor("b2", [NK * KK * KK], f32, kind="ExternalInput")
    out_d = nc.dram_tensor("out", [B_PER_CORE, NK, H, W], bf16,
                           kind="ExternalOutput")

    # Host-baked diagonal masks replicated over dw: masks3[p, dh, m, dw] = 1
    # iff p == m + dh.  Band matrices for a whole channel-group build as
    # A3[p, c, m, dw] = sum_dh masks3[p, dh, m, dw] * k[c, dh, dw] with five
    # wide TensorTensor ops (the dw-last layout keeps every operand's last
    # AP dim packed, so DVE runs them in 2x mode).
    masks_np = np.zeros((KP, KK, SH, KK), dtype=np.float32)
    for dh in range(KK):
        for m in range(SH):
            masks_np[m + dh, dh, m, :] = 1.0
    masks_d = nc.inline_tensor(masks_np.astype(ml_dtypes.bfloat16),
                               name="bandmasks")
    # strip-sum weights: drop the halo rows (compute ops must start at
    # partition 0, so pooling reads all 98 rows and the contraction masks)
    onesp_np = np.ones((KP, 1), dtype=np.float32)
    onesp_np[0, 0] = 0.0
    onesp_np[KP - 1, 0] = 0.0
    onesp_d = nc.inline_tensor(onesp_np, name="onesp")

    with tile.TileContext(nc) as tc, ExitStack() as ctx:
        xpool = ctx.enter_context(tc.tile_pool(name="xp", bufs=XB_X))
        dpool = ctx.enter_context(tc.tile_pool(name="dump", bufs=6))
        dpool_a = ctx.enter_context(tc.tile_pool(name="dumpa", bufs=2))
        cpool = ctx.enter_context(tc.tile_pool(name="const", bufs=1))
        mpool = ctx.enter_context(tc.tile_pool(name="mlp", bufs=2))
        apool = ctx.enter_context(tc.tile_pool(name="amat", bufs=3))
        tpool = ctx.enter_context(tc.tile_pool(name="atmp", bufs=2))
        cspool = ctx.enter_context(tc.tile_pool(name="csum", bufs=NS * NG + 2))
        opool = ctx.enter_context(tc.tile_pool(name="ostage", bufs=3))
        pp_ps = ctx.enter_context(
            tc.tile_pool(name="poolps", bufs=1, space=bass.MemorySpace.PSUM))
        kb_ps_pool = ctx.enter_context(
            tc.tile_pool(name="kbps", bufs=1, space=bass.MemorySpace.PSUM))
        cv_ps = ctx.enter_context(
            tc.tile_pool(name="convps", bufs=3, space=bass.MemorySpace.PSUM))

        # --- one-time constants ---
        # (tiles declared here; the DMAs are emitted after the first x group
        # so the x pipeline owns the DMA engines from t=0.  Earliest consts
        # use: onesp at the first strip-sum ~9us, the rest ~29us.)
        masks = cpool.tile([KP, KK, SH, KK], bf16)
        ones1 = cpool.tile([1, HID], f32)         # for partition broadcast
        nc.vector.memset(ones1[:], 1.0)
        onesp = cpool.tile([KP, 1], f32)          # strip-sum, halo rows zeroed
        # w1 is loaded once per channel group at partition base 0, so the h1
        # accumulation can consume each group's pooled means independently
        w1g = [cpool.tile([GC, HID], f32, name=f"w1g{g}") for g in range(NG)]
        b1row = cpool.tile([1, HID], f32)
        w2s = cpool.tile([HID, NK * KK * KK], f32)
        b2row = cpool.tile([1, NK * KK * KK], f32)

        def emit_const_loads_early():
            # after g0's x loads: 2 HWDGE slots; the DMA backlog absorbs it
            nc.sync.dma_start(onesp[:], onesp_d[:])
            nc.sync.dma_start(masks[:], masks_d[:])

        def emit_const_loads_late():
            # after g2's x loads, via Pool's SWDGE: the DMA FIFO slots these
            # behind the queued x transfers (they land ~29us, just before
            # the MLP consumes them) and the x pipeline's SP/HWDGE path is
            # never perturbed
            for g in range(NG):
                nc.gpsimd.dma_start(w1g[g][:], w1_d[g * GC:(g + 1) * GC, :])
            nc.gpsimd.dma_start(b1row[:], b1_d[:].unsqueeze(0))
            nc.gpsimd.dma_start(w2s[:], w2_d[:])
            nc.gpsimd.dma_start(b2row[:], b2_d[:].unsqueeze(0))

        # tiny warm-up matmul: absorbs the PE preamble wait + const DMA lane
        # ticks so real matmuls carry few semaphore waits (ISA slot limit).
        warm_ps = pp_ps.tile([1, 1], f32, tag="pool")
        nc.tensor.matmul(warm_ps[:], ones1[0:1, 0:1], ones1[0:1, 0:1],
                         start=True, stop=True)

        # x slots are zero-padded (cols 0/385) on first use; loads never
        # touch those columns, so slot rotation keeps them zero.  Halo rows
        # come from the host-side zero padding of x.
        x_acq = [0]

        def x_tile():
            t = xpool.tile([KP, GC, WP], bf16, tag="x")
            if x_acq[0] < XB_X:
                nc.gpsimd.memset(t[:, :, 0:1], 0.0)
                nc.gpsimd.memset(t[:, :, WP - 1:WP], 0.0)
            x_acq[0] += 1
            return t

        xt = {}      # (b, g, s) -> x tile
        csum = {}    # (b, g, s) -> [96, GC] f32 row sums

        def emit_load(b, g, split_last=False):
            """Load DMAs for one channel-group of sample b.  In the padded
            row coordinates, strip s covers rows [s*SH, s*SH + KP).  The
            head's very last tile is loaded as two half-group DMAs so its
            pooling overlaps the second half's transfer."""
            c0 = g * GC
            xsrc = x_d[b, c0:c0 + GC]
            for s in range(NS):
                t = x_tile()
                if split_last and s == NS - 1:
                    for h in range(2):
                        nc.sync.dma_start(
                            t[:, 4 * h:4 * h + 4, 1:W + 1],
                            xsrc[4 * h:4 * h + 4,
                                 s * SH:s * SH + KP, :].rearrange(
                                     "c r w -> r c w"))
                else:
                    nc.sync.dma_start(
                        t[:, :, 1:W + 1],
                        xsrc[:, s * SH:s * SH + KP, :].rearrange(
                            "c r w -> r c w"))
                xt[(b, g, s)] = t

        def pool_tile(b, g, s, split=None):
            """W-sums of one tile: tensor_scalar with the per-partition
            accumulator (160ns engine / ~255ns cadence per channel on DVE).
            Reads all 98 rows (partition offset must be 0); halo rows are
            dropped later by the onesp strip-sum contraction.  `split` maps
            engines to channel lists: in the head, DVE alone (2.04us/tile)
            cannot keep up with the 1.66us DMA cadence, so Pool and ACT
            each take a channel."""
            t = xt[(b, g, s)]
            cs = cspool.tile([KP, GC], f32, tag="cs")
            if split is None:
                split = [("dve", range(GC))]
            for eng, ccs in split:
                if not ccs:
                    continue
                if eng == "dve":
                    dump = dpool.tile([KP, W], bf16, tag="dump")
                    for cc in ccs:
                        nc.vector.tensor_scalar(
                            dump[:], t[:, cc, 1:W + 1], 1.0, 0.0,
                            op0=Alu.mult, op1=Alu.add,
                            accum_out=cs[:, cc:cc + 1])
                else:
                    dump = dpool_a.tile([KP, W], bf16, tag="dumpa",
                                        name="dumpa")
                    for cc in ccs:
                        nc.scalar.activation(dump[:], t[:, cc, 1:W + 1],
                                             Act.Copy,
                                             accum_out=cs[:, cc:cc + 1])
            csum[(b, g, s)] = cs

        def emit_mlp(b, warm=0):
            """Strip-sum matmuls + kernel-generator MLP (f32); returns kb.
            Chain only touches PE and ACT (DVE may be backlogged).  The
            strip-sums contract csum.T @ onesp so the channel means land as
            a COLUMN directly (no transpose round-trip).  For sample 0,
            pm-gated warm-up matmuls are interleaved into the chain's ACT
            gaps so PE stays continuously busy from the pm step through the
            first conv matmul -- by then the p-state ramp (3us) is done."""
            mlpx = mpool.tile([HID, 1 + NG], f32, tag="mlpx")
            h1s = mlpx[:, 0:1]
            pmg = mlpx[0:GC, 1:1 + NG]     # per-group pooled means, base 0
            pcol_ps = pp_ps.tile([GC, NG], f32, tag="pool")
            for g in range(NG):
                for s in range(NS):
                    # csum.T @ onesp: the group's channel sums land as a
                    # COLUMN at partition base 0 (no transpose round-trip)
                    nc.tensor.matmul(pcol_ps[0:GC, g:g + 1],
                                     csum[(b, g, s)][:], onesp[:],
                                     start=(s == 0), stop=(s == NS - 1))
                nc.scalar.activation(pmg[:, g:g + 1], pcol_ps[0:GC, g:g + 1],
                                     Act.Copy, scale=1.0 / (H * W))

            def warm2(n, cols=144):
                # p-state filler on the otherwise-idle kbps bank; gated on
                # the LAST group's means so it runs exactly in the chain's
                # ACT-hop gaps (not earlier, during the load phase).  Short
                # (144-col) fillers so an overshoot never delays the chain.
                if b != 0:
                    return
                for _ in range(n):
                    wp = kb_ps_pool.tile([1, NK * KK * KK], f32, tag="kbps",
                                         name="warm2")
                    nc.tensor.matmul(wp[:, 0:cols], pmg[0:1, NG - 1:NG],
                                     w2s[0:1, 0:cols], start=True, stop=True)

            h1_ps = pp_ps.tile([HID, 1], f32, tag="pool")
            for g in range(NG):
                nc.tensor.matmul(h1_ps[:], w1g[g][:], pmg[:, g:g + 1],
                                 start=(g == 0), stop=False)
            nc.tensor.matmul(h1_ps[:], b1row[:], ones1[0:1, 0:1],
                             start=False, stop=True)
            warm2(0)
            nc.scalar.activation(h1s, h1_ps[:], Act.Relu)

            k_ps = pp_ps.tile([1, NK * KK * KK], f32, tag="pool")
            nc.tensor.matmul(k_ps[:], h1s, w2s[:], start=True, stop=True)
            warm2(0)
            krow = mpool.tile([1, NK * KK * KK], f32, tag="krow")
            nc.scalar.activation(krow[:], k_ps[:], Act.Copy)

            kb = kb_ps_pool.tile([HID, NK * KK * KK], f32, tag="kbps")
            nc.tensor.matmul(kb[:], ones1[:], krow[:], start=True, stop=False)
            nc.tensor.matmul(kb[:], ones1[:], b2row[:], start=False, stop=True)
            kbs = mpool.tile([HID, NK * KK * KK], bf16, tag="kbs")
            # group-0's slice first: the first band build gates on only this
            nc.scalar.activation(kbs[:, 0:GC * KK * KK],
                                 kb[:, 0:GC * KK * KK], Act.Copy)
            nc.scalar.activation(kbs[:, GC * KK * KK:],
                                 kb[:, GC * KK * KK:], Act.Copy)
            # gated warm-up: PE idled through pooling, so its p-state clock
            # dropped; back-to-back dummy matmuls (first ready only once
            # kbs exists) ramp it back to full speed exactly while DVE builds
            # the first band matrices, so conv starts at 2.4 GHz.  Sample 1's
            # MLP runs mid-conv0 with PE already hot, so warm=0 there.
            for _ in range(warm):
                w_ps = pp_ps.tile([1, NK * KK * KK], f32, tag="pool",
                                  name="w_ps")
                nc.tensor.matmul(w_ps[:], kbs[0:1, 0:1], kbs[0:1, :],
                                 start=True, stop=True)
            return kbs

        def emit_amat_group(kbs, g, halves=1):
            """Band matrices for a whole 8-channel group in 5 wide DVE
            TensorTensor ops (2x mode: every operand's last AP dim is the
            packed dw axis): A3[p, c, m, dw] = sum_dh masks3 * k[c, dh, dw].
            halves=2 splits the build in two for lower first-channel latency.
            """
            a3 = apool.tile([KP, GC, SH, KK], bf16, tag="a3", name="a3")
            # [98, 8ch, 9] view of this group's kernel block
            gb = kbs[0:KP, g * GC * 9:(g + 1) * GC * 9].rearrange(
                "p (c z) -> p c z", c=GC)
            hc = GC // halves
            for h in range(halves):
                c0, c1 = h * hc, (h + 1) * hc
                for dh in range(KK):
                    m_b = masks[:, dh].unsqueeze(1).broadcast_to(
                        [KP, hc, SH, KK])
                    k_b = gb[:, c0:c1, dh * KK:(dh + 1) * KK].unsqueeze(2) \
                        .broadcast_to([KP, hc, SH, KK])
                    if dh == 0:
                        nc.vector.tensor_tensor(a3[:, c0:c1], m_b, k_b,
                                                Alu.mult)
                    else:
                        t = tpool.tile([KP, hc, SH, KK], bf16, tag="at",
                                       name="at")
                        nc.vector.tensor_tensor(t[:], m_b, k_b, Alu.mult)
                        nc.vector.tensor_tensor(a3[:, c0:c1], a3[:, c0:c1],
                                                t[:], Alu.add)
            return a3

        def emit_conv_channel(b, c, a3, ob):
            """12 PSUM-accumulated matmuls + drains for channel c.
            dw is the OUTER loop within a strip pair so the band-matrix
            weight slice is reused by two consecutive matmuls: the PE
            sequencer emits half the Ldweights (interleaved accumulation on
            two PSUM banks is fine -- accumulation state lives in PSUM)."""
            g, cc = divmod(c, GC)
            last = (b == B_PER_CORE - 1 and c == NK - 1)
            for j in range(NS // 2):
                o_ps = cv_ps.tile([SH, 2, 512], f32, tag="cv")  # 2 banks
                for dw in range(KK):
                    for half in range(2):
                        s = 2 * j + half
                        t = xt[(b, g, s)]
                        nc.tensor.matmul(o_ps[:, half, 0:W],
                                         a3[:, cc, :, dw],
                                         t[:, cc, dw:dw + W],
                                         start=(dw == 0), stop=(dw == KK - 1))
                # PSUM drains all on ACT (GpSimd cannot access PSUM on
                # HW; DVE carries every band-matrix build and all pooling)
                if last:
                    # final channel of the kernel: drain and store per STRIP
                    # so the critical tail is one [96,384] drain + one small
                    # store.  Strip 2's drain runs on DVE so ACT's in-order
                    # queue reaches the final strip-3 drain immediately, and
                    # the last three stores go out on three DIFFERENT issue
                    # paths (Pool SWDGE / SP HWDGE / ACT HWDGE) so their
                    # descriptor generation runs in parallel.
                    engs = [nc.gpsimd, nc.gpsimd, nc.sync, nc.scalar]
                    for half in range(2):
                        s = 2 * j + half
                        drain = nc.vector.tensor_scalar if s == 2 else None
                        if drain is not None:
                            nc.vector.tensor_scalar(
                                ob[:, c % 2, s:s + 1, :],
                                o_ps[:, half:half + 1, 0:W],
                                1.0, 0.0, op0=Alu.mult, op1=Alu.add)
                        else:
                            nc.scalar.activation(ob[:, c % 2, s:s + 1, :],
                                                 o_ps[:, half:half + 1, 0:W],
                                                 Act.Copy)
                        engs[s].dma_start(
                            out_d[b, c, s * SH:(s + 1) * SH, :].rearrange(
                                "(s2 p) w -> p s2 w", s2=1),
                            ob[:, c % 2, s:s + 1])
                else:
                    nc.scalar.activation(ob[:, c % 2, 2 * j:2 * j + 2, :],
                                         o_ps[:, :, 0:W], Act.Copy)
            # one store per channel, issued from the idle Pool engine's
            # SWDGE path: 384 descriptors per store keeps two stores inside
            # the 1024-descriptor SWDGE ring, so issue never blocks on a
            # prior transfer, and the tail after the last drain is short
            if not last:
                nc.gpsimd.dma_start(
                    out_d[b, c, :, :].rearrange("(s p) w -> p s w", s=NS),
                    ob[:, c % 2])

        # ---------------- schedule ----------------
        # head: load + pool sample 0.  ALL pooling runs on DVE (1.28us per
        # tile vs the 1.66us DMA cadence, so DVE keeps pace): ACT's in-order
        # queue stays empty so the MLP chain's ACT hops run the moment each
        # group's strip-sums land.
        for g in range(NG):
            emit_load(0, g, split_last=(g == NG - 1))
            if g == 0:
                emit_const_loads_early()
            elif g == 1:
                # early enough that their DMA-FIFO slots interleave with the
                # head x transfers: every const lands (+sem) by ~27us, ahead
                # of the MLP chain's first use at ~30.5us
                emit_const_loads_late()
            for s in range(NS):
                if (g, s) == (NG - 1, NS - 1):
                    # very last tile: first half pools on DVE while the
                    # second half transfers; ACT (idle since ~26us) takes
                    # two channels of the second half
                    pool_tile(0, g, s, split=[("dve", range(6)),
                                              ("act", [6, 7])])
                elif (g, s) == (NG - 1, NS - 2):
                    # all-DVE so DVE reaches the last tile's pools promptly
                    pool_tile(0, g, s)
                else:
                    pool_tile(0, g, s, split=[("dve", range(6)),
                                              ("act", [6, 7])])
        kb0 = emit_mlp(0)

        # conv sample 0; sample 1's loads go out immediately in order
        # g2,g3,g0,g1 (g2/g3 land on fresh slots, g0/g1 self-gate on slot
        # reuse after conv0's matmuls release the sample-0 slots).
        for g in (2, 3, 0, 1):
            emit_load(1, g)

        _ob_cur = {}

        def _conv_ch(b, c, a3):
            if c % 2 == 0:
                _ob_cur["t"] = opool.tile([SH, 2, NS, W], bf16, tag="ob",
                                          name="ob")
            emit_conv_channel(b, c, a3, _ob_cur["t"])

        def conv_group(b, kbs, g, halves=1):
            a3 = emit_amat_group(kbs, g, halves=halves)
            for cc in range(GC):
                _conv_ch(b, g * GC + cc, a3)

        # Band-matrix builds are the latency-critical DVE work (each conv
        # group's a3 must exist before PE reaches that group), so they are
        # emitted EAGERLY -- back-to-back where slots allow (apool holds 3)
        # -- and sample 1's pooling fills the DVE slack between them.
        # MLP1 is emitted BETWEEN conv0-g2 and conv0-g3: PE runs it
        # mid-conv with the p-state hot, and DVE builds sample 1's first
        # band matrices during conv0-g3 -- no PE bubble between samples.
        a3_01 = [emit_amat_group(kb0, 0, halves=8),
                 emit_amat_group(kb0, 1, halves=2)]
        for cc in range(GC):
            _conv_ch(0, 0 * GC + cc, a3_01[0])
        for s in range(NS):
            pool_tile(1, 2, s)
        a3_2 = emit_amat_group(kb0, 2, halves=2)
        for cc in range(GC):
            _conv_ch(0, 1 * GC + cc, a3_01[1])
        for s in range(NS):
            pool_tile(1, 3, s)
        a3_3 = emit_amat_group(kb0, 3, halves=2)
        for cc in range(GC):
            _conv_ch(0, 2 * GC + cc, a3_2)
        for g in (0, 1):
            for s in range(NS):
                pool_tile(1, g, s)
        kb1 = emit_mlp(1, warm=0)
        for cc in range(GC):
            _conv_ch(0, 3 * GC + cc, a3_3)
        for g in range(NG):
            conv_group(1, kb1, g, halves=2)

    nc.compile()
    return nc


def _make_exec():
    """Build + jit the SPMD executable once; returns a callable over numpy inputs."""
    import jax
    from jax.sharding import Mesh, PartitionSpec
    from jax.experimental.shard_map import shard_map
    from concourse import bass2jax
    import concourse.mybir as mybir

    nc = _build_nc()
    _CACHE["nc"] = nc
    bass2jax.install_neuronx_cc_hook()

    in_names, out_names, out_shapes, out_dtypes = [], [], [], []
    for alloc in nc.m.functions[0].allocations:
        if not isinstance(alloc, mybir.MemoryLocationSet):
            continue
        name = alloc.memorylocations[0].name
        if alloc.kind == "ExternalInput":
            in_names.append(name)
        elif alloc.kind == "ExternalOutput":
            out_names.append(name)
            out_shapes.append(tuple(alloc.tensor_shape))
            out_dtypes.append(mybir.dt.np(alloc.dtype))
    partition_name = nc.partition_id_tensor.name if nc.partition_id_tensor else None
    if partition_name in in_names:
        in_names.remove(partition_name)
    n_params = len(in_names)
    out_avals = [jax.core.ShapedArray(s, d) for s, d in zip(out_shapes, out_dtypes)]
    all_names = in_names + out_names
    if partition_name is not None:
        all_names = all_names + [partition_name]
    donate = tuple(range(n_params, n_params + len(out_names)))

    def _body(*args):
        operands = list(args)
        if partition_name is not None:
            operands.append(bass2jax.partition_id_tensor())
        outs = bass2jax._bass_exec_p.bind(
            *operands,
            out_avals=tuple(out_avals),
            in_names=tuple(all_names),
            out_names=tuple(out_names),
            lowering_input_output_aliases=(),
            sim_require_finite=True,
            sim_require_nnan=True,
            nc=nc,
        )
        return tuple(outs)

    devices = jax.devices()[:N_CORES]
    mesh = Mesh(np.asarray(devices), ("core",))
    in_specs = (PartitionSpec("core"),) * (n_params + len(out_names))
    out_specs = (PartitionSpec("core"),) * len(out_names)
    sharded = jax.jit(
        shard_map(_body, mesh=mesh, in_specs=in_specs, out_specs=out_specs,
                  check_rep=False),
        donate_argnums=donate, keep_unused=True)

    def run(in_maps):
        concat_in = [
            np.concatenate([np.asarray(in_maps[c][nm]) for c in range(N_CORES)], axis=0)
            for nm in in_names
        ]
        concat_zeros = [
            np.zeros((N_CORES * s[0], *s[1:]), d)
            for s, d in zip(out_shapes, out_dtypes)
        ]
        out_arrs = sharded(*concat_in, *concat_zeros)
        out_arrs = jax.block_until_ready(out_arrs)
        return {nm: np.asarray(out_arrs[i]) for i, nm in enumerate(out_names)}

    return run


def _run(inputs, trace=False):
    import ml_dtypes
    if "exec" not in _CACHE:
        _CACHE["exec"] = _make_exec()
    run = _CACHE["exec"]

    x16 = np.ascontiguousarray(inputs["x"]).astype(ml_dtypes.bfloat16)
    # one zero row of padding top and bottom: edge strips load their halo
    # rows straight from HBM like every other strip
    x16 = np.pad(x16, ((0, 0), (0, 0), (1, 1), (0, 0)))
    in_maps = []
    for i in range(N_CORES):
        in_maps.append({
            "x": x16[i * B_PER_CORE:(i + 1) * B_PER_CORE],
            "w1": inputs["w1"], "b1": inputs["b1"],
            "w2": inputs["w2"], "b2": inputs["b2"],
        })
    outs = run(in_maps)
    out = outs["out"].reshape(B, NK, H, W).astype(np.float32)
    return out, None


def kernel(**inputs):
    out, _ = _run(inputs, trace=False)
    return out

